# revision 35
# baseline (speedup 1.0000x reference)
"""Trainium2 Bass kernel for the 14-term hydrogen-orbital basis evaluation.

Computes out[i,j] = sum_k coeffs[k] * R_{n_k l_k}(r) * Y_{l_k m_k}(theta, phi)
for position (2048, 4096, 3) = (r, theta, phi), pure data-parallel across
8 NeuronCores (256 rows of OutN each).

v4 structure (81.7us sim vs the 94us baseline):
  * 7 ACT lookups per block instead of 9:
      - S3 = sin(phi+d3) eliminated: the (l=1,n=3) m=+-1 pair shares one
        (x,y) phase vector across its two radial monomials
        (R31 ~ rE3 - r^2E3/6), so the r^2E3 part reuses S2 with PE
        coefficient -ro2/6.
      - S4 = sin(phi+d4) eliminated: first-harmonic phase sins span a 2D
        space, so ro4*sin(phi+d4) = A*S1 + B*S2 and the (l=2,m=+-1) term
        becomes A*(Qu*sS1) + B*(Qu*sS2) - same product count, one fewer
        table pass.  (Falls back to an S4 lookup if |sin(d1-d2)| is
        small.)
  * 18 fp16 elementwise products, ~14 passes on DVE (0.52 ns/col) and 4
    PE-terminal sites (E2sq, K1, Quu, Z1 - products no DVE op consumes)
    on Pool (1.98 ns/col), so Pool latency never gates the DVE stream.
  * 15 constant-weighted accumulations on PE via diagonal [128,128]
    fp16 weights into PSUM.  Block b's matmuls are emitted after block
    b+1's products (pe_lag), so PE executes each block as one gapless
    term-outer batch and holds its fast p-state.  PE terms whose
    maximum possible contribution is below 5e-4 (vs output absmax
    ~6e-2) are dropped (costs ~7e-3 relative error, tolerance 2e-2).
  * PSUM evictions on ACT (which has slack); PSUM pair tiles let one
    evict instruction cover two banks.
  * Each block's r-plane and theta|phi planes live in separate tiles
    with separate DMAs, so E2/E3 depend only on the small r transfer.
    Block0's planes, the diagonal weights, and the activation bias
    constants ship inside one host-built fp16 input plane, fetched at
    t=0 by three parallel DMA channels (SP / ACT HWDGE / Pool SWDGE) -
    no on-device preamble, first lookup after a ~0.4us payload.
  * Out-DMAs are emitted two blocks late (evicts one block late) so
    the in-order SP queue never parks a pending out-DMA in front of
    the next input slab fetch.

Environment notes: this container's walrus rejects the CUSTOM_DVE_ANT
extension and Pool scalar_tensor_tensor, so only stock engine ops are
used.  Hardware CTRL/DMA instructions accept a single sync wait, so a
BIR post-pass splits Tile's multi-wait tail drain onto NoOps
(_split_excess_waits).
"""

import math

import numpy as np

COLS = 8192  # per-core elements per partition row: 256*4096/128
P = 128
N_CORES = 8
PS_CHUNK = 512  # PSUM bank chunk (matmul moving-dim max)

_BUILD_CACHE = {}
LAST_RESULTS = None

# PE terms in emission (and weight-layout) order: roughly by the time the
# term tensor is produced so PE never waits on a late term.
PE_TERMS = [
    "E2", "E3", "P2", "P3", "E2sq", "Q3", "Pu2", "Pu3",
    "K1", "Qu", "K2", "K34", "Z1", "Quu", "Z2", "K5",
]

# max |tensor| over the r,theta,phi in [0,1) domain, for term dropping
E1 = math.exp(-0.5)
E13 = math.exp(-1.0 / 3.0)
S1MAX = math.sin(1.0)
TERM_MAX = {
    "E2": 1.0, "E3": 1.0, "E2sq": 1.0, "P2": E1, "P3": E13, "Q3": E13,
    "Pu2": E1, "Pu3": E13, "Qu": E13, "Quu": E13,
    "K1": E1 * S1MAX, "K2": E13 * S1MAX, "K34": E13 * S1MAX,
    "Z1": E13 * S1MAX, "Z2": E13 * S1MAX, "K5": E13 * S1MAX * S1MAX,
}
DROP_BOUND = 5e-4

# bias constant layout (columns appended after the 16*128 weight columns)
BIAS_NAMES = ["zero", "hpi", "d1", "d2", "d4", "d5"]

# Pool takes PE-terminal products only (no DVE consumers), so Pool latency
# never gates the DVE stream; cross-block pipelining keeps Pool busy.
_MID_POOL = {"E2sq": 1.0, "K1": 1.0, "Quu": 1.0, "Z1": 1.0}

DEFAULT_CFG = {
    "blk_sizes": [768, 2048, 2048, 2048, 864, 416],
    "pool_plan": [
        {"E2sq": 1.0, "K1": 1.0, "Quu": 1.0},
        _MID_POOL, _MID_POOL, _MID_POOL,
        {"E2sq": 1.0, "K1": 1.0, "Quu": 1.0},
        {"E2sq": 1.0, "K1": 1.0, "Quu": 1.0},
    ],
    # per-block number of column sub-splits for the product/PE stream:
    # products are emitted per-half and PE's matmuls chase each half, so PE
    # runs long uninterrupted batches (p-state) while staying one half
    # behind the producers.
    "halves": [1, 1, 1, 1, 1, 1],
    "pe_lag": 1,
    "pair_ch": 2,  # PSUM banks covered by one evict instruction
    "evict": "a",  # per-chunk-group engine pattern, a=ACT v=DVE
    "wk_bufs": 40,
    "io_bufs": 2,
    "prefetch": 1,
    "rot_terms": 0,  # rotating PE batches to start late measured worse
    "out_q": "sp",
}


# --------------------------------------------------------------------------
# host-side math: fold the 14 coeffs into the flat term weights + sin phases
# --------------------------------------------------------------------------
def _derive_terms(coeffs):
    c = np.asarray(coeffs, dtype=np.float64)

    def rad_norm(n, l):
        return math.sqrt(
            (2.0 / n) ** 3
            * math.factorial(n - l - 1)
            / (2.0 * n * math.factorial(n + l))
        )

    def sph_norm(l, m):
        am = abs(m)
        return math.sqrt(
            (2 * l + 1)
            / (4.0 * math.pi)
            * math.factorial(l - am)
            / math.factorial(l + am)
        )

    n10, n20, n21, n30, n31, n32 = (
        rad_norm(*p) for p in [(1, 0), (2, 0), (2, 1), (3, 0), (3, 1), (3, 2)]
    )
    k00, k10, k11, k20, k21, k22 = (
        sph_norm(*p) for p in [(0, 0), (1, 0), (1, 1), (2, 0), (2, 1), (2, 2)]
    )
    s2 = math.sqrt(2.0)
    g32 = n32 * 4.0 / 9.0  # R32 = g32 * r^2 * E3
    G = c[11] * k20 * g32

    W = {}
    W["a1"] = k00 * c[0] * n10          # E2sq
    W["a2"] = k00 * c[1] * n20 * 2.0    # E2
    W["a3"] = -k00 * c[1] * n20         # P2
    W["a4"] = k00 * c[5] * n30 * 3.0    # E3
    W["a5"] = -k00 * c[5] * n30 * 2.0   # P3
    W["c0q"] = k00 * c[5] * n30 * 2.0 / 9.0 - G / 2.0  # Q3
    W["w11"] = k10 * c[3] * n21         # Pu2
    W["w12"] = k10 * c[7] * n31 * 8.0 / 3.0  # Pu3
    W["c1"] = -k10 * c[7] * n31 * 4.0 / 9.0  # Qu
    W["c2"] = 1.5 * G                   # Quu

    x1 = -s2 * k11 * n21 * c[2]
    y1 = -s2 * k11 * n21 * c[4]
    x2 = -s2 * k11 * n31 * 8.0 / 3.0 * c[6]
    y2 = -s2 * k11 * n31 * 8.0 / 3.0 * c[8]
    x4 = -3.0 * s2 * k21 * g32 * c[10]
    y4 = -3.0 * s2 * k21 * g32 * c[12]
    z1 = 3.0 * s2 * k22 * g32 * c[9]
    z2 = 3.0 * s2 * k22 * g32 * c[13]

    def fold(x, y, span):
        # x*sin(t) + y*cos(t) = rho*sin(t + d); keep t+d inside the ACT Sin
        # table's [-pi, pi] domain for t in [0, span] by flipping by pi.
        rho = math.hypot(x, y)
        d = math.atan2(y, x)
        if d + span > math.pi:
            d -= math.pi
            rho = -rho
        return rho, d

    W["ro1"], W["d1"] = fold(x1, y1, 1.0)   # K1 = P2*s*S1
    W["ro2"], W["d2"] = fold(x2, y2, 1.0)   # K2 = P3*s*S2
    # (l=1,n=3) r^2E3 part shares S2: rho3*sin(phi+d3) == -(ro2/6)*sin(phi+d2)
    W["ro3"] = -W["ro2"] / 6.0              # K34 = Q3*s*S2
    W["ro4"], W["d4"] = fold(x4, y4, 1.0)   # (l=2,m=+-1) phase pair
    W["ro5"], W["d5"] = fold(z1, z2, 2.0)   # K5 = Q3*s*sS5

    # express ro4*sin(phi+d4) over the {S1, S2} basis when well conditioned
    den = math.sin(W["d1"] - W["d2"])
    W["s4_basis"] = abs(den) >= 0.2
    if W["s4_basis"]:
        W["zA"] = W["ro4"] * math.sin(W["d4"] - W["d2"]) / den
        W["zB"] = W["ro4"] * math.sin(W["d1"] - W["d4"]) / den
    else:  # fallback: keep the S4 lookup
        W["zA"] = W["zB"] = 0.0
    return {k: (float(v) if not isinstance(v, bool) else v) for k, v in W.items()}


def _pe_coef(W):
    coef = {
        "E2": W["a2"], "E3": W["a4"], "E2sq": W["a1"], "P2": W["a3"],
        "P3": W["a5"], "Q3": W["c0q"], "Pu2": W["w11"], "Pu3": W["w12"],
        "Qu": W["c1"], "Quu": W["c2"], "K1": W["ro1"], "K2": W["ro2"],
        "K34": W["ro3"], "K5": W["ro5"],
    }
    if W["s4_basis"]:
        coef["Z1"] = W["zA"]
        coef["Z2"] = W["zB"]
    else:
        coef["Z1"] = W["ro4"]  # Z1 becomes Qu*sS4 in the fallback build
        coef["Z2"] = 0.0
    return coef


def _live_terms(W):
    coef = _pe_coef(W)
    live = [
        n for n in PE_TERMS
        if abs(coef[n]) * TERM_MAX[n] >= DROP_BOUND
    ]
    return live, coef


def _build_wts_plane(W):
    """Host-built fp16 plane [P, 16*128 + 128]: up to 16 diagonal coef*I
    weight matrices (live-term layout order) followed by one 128-col block
    whose first columns are the activation bias constants (each replicated
    down the partition dim)."""
    live, coef = _live_terms(W)
    wts = np.zeros((P, 17 * P), dtype=np.float16)
    for i, name in enumerate(live):
        blk = wts[:, i * P : (i + 1) * P]
        np.fill_diagonal(blk, np.float16(coef[name]))
    bias_vals = {
        "zero": 0.0,
        "hpi": math.pi / 2.0,
        "d1": W["d1"],
        "d2": W["d2"],
        "d4": W["d4"],
        "d5": W["d5"],
    }
    for j, nm in enumerate(BIAS_NAMES):
        wts[:, 16 * P + j] = np.float16(bias_vals[nm])
    return wts


# --------------------------------------------------------------------------
# BIR post-pass: hardware allows a single sync-wait per instruction; Tile's
# tail drain can carry several.  Split extras onto preceding same-engine NoOps.
# --------------------------------------------------------------------------
def _split_excess_waits(nc, max_waits=1):
    import concourse.mybir as mybir

    for bb in nc.m.functions[0].blocks:
        insts = bb.instructions
        i = 0
        while i < len(insts):
            inst = insts[i]
            si = getattr(inst, "sync_info", None)
            waits = list(si.on_wait) if si is not None and si.on_wait else []
            if len(waits) > max_waits:
                keep = waits[:max_waits]
                extra = waits[max_waits:]
                chunks = [
                    extra[j : j + max_waits] for j in range(0, len(extra), max_waits)
                ]
                new_insts = []
                for ci, ch in enumerate(chunks):
                    nop = mybir.InstNoOp(
                        name=f"{inst.name}-wsplit-{ci}",
                        engine=inst.engine,
                        ins=[],
                        outs=[],
                        sync_info=mybir.SyncInfo(on_wait=ch, on_update=[]),
                    )
                    nc.register_instruction(nop, overwrite=True)
                    new_insts.append(nop)
                inst.sync_info = mybir.SyncInfo(
                    on_wait=keep,
                    on_update=list(si.on_update) if si.on_update else [],
                )
                for k, ni in enumerate(new_insts):
                    insts.insert(i + k, ni)
                i += len(new_insts)
            i += 1


# --------------------------------------------------------------------------
# kernel builder
# --------------------------------------------------------------------------
def _build_nc(W, cfg=None):
    import concourse.bass as bass
    import concourse.mybir as mybir
    from concourse import tile

    AF = mybir.ActivationFunctionType
    f32 = mybir.dt.float32
    f16 = mybir.dt.float16
    MULT = mybir.AluOpType.mult

    cfg = {**DEFAULT_CFG, **(cfg or {})}
    blk_sizes = list(cfg["blk_sizes"])
    assert sum(blk_sizes) == COLS
    NB = len(blk_sizes)
    blk_offs = [sum(blk_sizes[:i]) for i in range(NB)]
    f_blk = max(blk_sizes)
    pool_plan = cfg["pool_plan"]
    assert len(pool_plan) == NB
    halves = cfg["halves"]
    assert len(halves) == NB
    pair_ch = cfg["pair_ch"]
    evict = cfg["evict"]
    s4_basis = W["s4_basis"]

    live, _coef = _live_terms(W)

    nc = bass.Bass()
    b0 = blk_sizes[0]
    pos = nc.dram_tensor("pos", [P, 3 * (COLS - b0)], f16, kind="ExternalInput")
    wts_d = nc.dram_tensor("wts", [P, 3 * b0 + 17 * P], f16,
                           kind="ExternalInput")
    out_d = nc.dram_tensor("out", [P, COLS], f16, kind="ExternalOutput")

    with tile.TileContext(nc) as tc:
        with (
            tc.tile_pool(name="wt", bufs=1) as wtp,
            tc.tile_pool(name="ior", bufs=cfg["io_bufs"]) as ior,
            tc.tile_pool(name="iot", bufs=cfg["io_bufs"]) as iot,
            tc.tile_pool(name="ot", bufs=3) as otp,
            tc.tile_pool(name="wk", bufs=cfg["wk_bufs"]) as wk,
            tc.tile_pool(name="ps", bufs=8 // pair_ch, space="PSUM") as ps,
        ):
            slabs_r = {}
            slabs_tp = {}

            def fetch(b):
                if b >= NB or b == 0:
                    return
                cf, sz = blk_offs[b] - b0, blk_sizes[b]
                # r-plane and theta|phi planes in separate tiles with
                # separate DMAs: the block's E2/E3 lookups depend only on
                # the small r transfer.
                sr = ior.tile([P, f_blk], f16, tag="slr", name=f"slr{b}")
                nc.sync.dma_start(sr[:, 0:sz], pos[:, 3 * cf : 3 * cf + sz])
                st = iot.tile([P, 2 * f_blk], f16, tag="slt", name=f"slt{b}")
                nc.sync.dma_start(
                    st[:, 0 : 2 * sz], pos[:, 3 * cf + sz : 3 * (cf + sz)]
                )
                slabs_r[b] = sr
                slabs_tp[b] = st

            # block0's r / theta|phi planes and the weight plane ride at the
            # head of the wts dram tensor and are fetched by THREE parallel
            # HWDGE queues (SP / ACT / DVE), so the first lookup starts after
            # only the tiny r transfer (~0.4us payload).
            sr0 = ior.tile([P, f_blk], f16, tag="slr", name="slr0")
            nc.sync.dma_start(sr0[:, 0:b0], wts_d[:, 0:b0])
            st0 = iot.tile([P, 2 * f_blk], f16, tag="slt", name="slt0")
            nc.scalar.dma_start(st0[:, 0 : 2 * b0], wts_d[:, b0 : 3 * b0])
            slabs_r[0] = sr0
            slabs_tp[0] = st0
            wt = wtp.tile([P, 17 * P], f16, tag="wt", name="wt")
            nc.gpsimd.dma_start(wt[:, :], wts_d[:, 3 * b0 :])
            diag = {name: wt[:, i * P : (i + 1) * P] for i, name in enumerate(live)}
            bias_ap = {
                nm: wt[:, 16 * P + j : 16 * P + j + 1]
                for j, nm in enumerate(BIAS_NAMES)
            }

            def produce(b):
                sz = blk_sizes[b]
                slr = slabs_r.pop(b)
                slt = slabs_tp.pop(b)
                rT = slr[:, 0:sz]
                thT = slt[:, 0:sz]
                phT = slt[:, sz : 2 * sz]
                t = {}
                pool_frac = pool_plan[b]

                def T(tagname):
                    tl = wk.tile([P, f_blk], f16, tag="wk", name=f"{tagname}{b}")
                    ap = tl[:, 0:sz]
                    t[tagname] = ap
                    return ap

                # ---- ACT lookups (fp16 in -> fp16 out) ----
                nc.scalar.activation(T("E2"), rT, AF.Exp, scale=-0.5,
                                     bias=bias_ap["zero"])
                nc.scalar.activation(T("E3"), rT, AF.Exp,
                                     scale=float(np.float32(-1.0 / 3.0)),
                                     bias=bias_ap["zero"])
                nc.scalar.activation(T("u"), thT, AF.Sin, bias=bias_ap["hpi"])
                nc.scalar.activation(T("s"), thT, AF.Sin, bias=bias_ap["zero"])
                nc.scalar.activation(T("S1"), phT, AF.Sin, bias=bias_ap["d1"])
                nc.scalar.activation(T("S2"), phT, AF.Sin, bias=bias_ap["d2"])
                if not s4_basis:
                    nc.scalar.activation(T("S4"), phT, AF.Sin, bias=bias_ap["d4"])
                nc.scalar.activation(T("S5"), phT, AF.Sin, bias=bias_ap["d5"],
                                     scale=2.0)

                # ---- products (fp16 TT on DVE, column-split to Pool) ----
                def TT(site, a, b_, c0, c1):
                    out = t[site] if site in t else T(site)
                    w = c1 - c0
                    pf = pool_frac.get(site, 0.0)
                    pc = min(w, int(round(pf * w / 16.0)) * 16)
                    af = pool_frac.get(site + "_act", 0.0)
                    ac = min(w - pc, int(round(af * w / 16.0)) * 16)
                    if pc > 0:
                        nc.gpsimd.tensor_tensor(
                            out[:, c0 : c0 + pc], a[:, c0 : c0 + pc],
                            b_[:, c0 : c0 + pc], MULT
                        )
                    if ac > 0:
                        # squares only: out = a*a via the ACT Square table
                        nc.scalar.activation(
                            out[:, c0 + pc : c0 + pc + ac],
                            a[:, c0 + pc : c0 + pc + ac],
                            AF.Square, bias=bias_ap["zero"],
                        )
                    if pc + ac < w:
                        nc.vector.tensor_tensor(
                            out[:, c0 + pc + ac : c1], a[:, c0 + pc + ac : c1],
                            b_[:, c0 + pc + ac : c1], MULT
                        )
                    return out

                def products(c0, c1):
                    # ordered by input readiness (ACT lookup order), with
                    # Pool-assigned terminal sites placed where their inputs
                    # exist; within-engine order stays topological.
                    TT("P2", rT, t["E2"], c0, c1)
                    TT("E2sq", t["E2"], t["E2"], c0, c1)
                    TT("P3", rT, t["E3"], c0, c1)
                    TT("Q3", rT, t["P3"], c0, c1)
                    TT("Pu2", t["P2"], t["u"], c0, c1)
                    TT("Pu3", t["P3"], t["u"], c0, c1)
                    TT("Qu", t["Q3"], t["u"], c0, c1)
                    TT("sS1", t["s"], t["S1"], c0, c1)
                    TT("K1", t["P2"], t["sS1"], c0, c1)
                    TT("Quu", t["Qu"], t["u"], c0, c1)
                    TT("sS2", t["s"], t["S2"], c0, c1)
                    TT("K2", t["P3"], t["sS2"], c0, c1)
                    TT("K34", t["Q3"], t["sS2"], c0, c1)
                    if s4_basis:
                        TT("Z1", t["Qu"], t["sS1"], c0, c1)
                        TT("Z2", t["Qu"], t["sS2"], c0, c1)
                    else:
                        TT("sS4", t["s"], t["S4"], c0, c1)
                        TT("Z1", t["Qu"], t["sS4"], c0, c1)
                    TT("sS5", t["s"], t["S5"], c0, c1)
                    TT("ssS5", t["s"], t["sS5"], c0, c1)
                    TT("K5", t["Q3"], t["ssS5"], c0, c1)

                # ---- PE: diagonal-weight matmuls accumulate into PSUM ----
                # term-outer within each half so PE gets long uninterrupted
                # runs (p-state ramp) while staying one half behind the
                # producers.
                nch = (sz + PS_CHUNK - 1) // PS_CHUNK
                grp = []
                k = 0
                while k < nch:
                    g = min(pair_ch, nch - k)
                    pst = ps.tile(
                        [P, pair_ch * PS_CHUNK], f32, tag="ps", name=f"ps{b}_{k}"
                    )
                    grp.append((k, g, pst))
                    k += g
                terms = [nm for nm in live if nm != "Z2" or s4_basis]
                n = len(terms)
                if cfg["rot_terms"]:
                    # start each PE batch on the last-produced term so the
                    # batch begins only when every term is ready and then
                    # streams without stalls; start/stop flags follow
                    # emission order (the sum is commutative).
                    emit_order = [terms[-1]] + terms[:-1]
                else:
                    emit_order = terms

                def matmuls(c0, c1):
                    for i, name in enumerate(emit_order):
                        for k0, g, pst in grp:
                            for j in range(g):
                                lo = (k0 + j) * PS_CHUNK
                                if lo >= min(sz, c1) or lo + PS_CHUNK <= c0:
                                    continue
                                ln = min(PS_CHUNK, sz - lo, c1 - lo)
                                nc.tensor.matmul(
                                    pst[:, j * PS_CHUNK : j * PS_CHUNK + ln],
                                    diag[name],
                                    t[name][:, lo : lo + ln],
                                    start=(i == 0),
                                    stop=(i == n - 1),
                                )

                if cfg.get("pe_lag"):
                    # emit products only; the caller emits this block's
                    # matmuls after the NEXT block's products, so PE runs
                    # each block as one gapless batch.
                    products(0, sz)
                    return grp, lambda: matmuls(0, sz)
                nh = halves[b]
                cuts = [0]
                for h in range(1, nh):
                    cuts.append(((sz * h // nh) // PS_CHUNK) * PS_CHUNK)
                cuts.append(sz)
                for h in range(nh):
                    products(cuts[h], cuts[h + 1])
                    matmuls(cuts[h], cuts[h + 1])
                return grp, None

            ev_ctr = [0]
            ots = {}

            def evict_blk(b, grp):
                sz = blk_sizes[b]
                ot = otp.tile([P, f_blk], f16, tag="ot", name=f"ot{b}")
                ots[b] = ot
                for k0, g, pst in grp:
                    lo = k0 * PS_CHUNK
                    ln = min(g * PS_CHUNK, sz - lo)
                    kk = ev_ctr[0]
                    ev_ctr[0] += 1
                    if evict[kk % len(evict)] == "a":
                        nc.scalar.activation(
                            ot[:, lo : lo + ln], pst[:, 0:ln], AF.Copy
                        )
                    else:
                        nc.vector.tensor_copy(ot[:, lo : lo + ln], pst[:, 0:ln])

            def flush(b):
                sz = blk_sizes[b]
                cf0 = blk_offs[b]
                ot = ots.pop(b)
                q = {"sp": nc.sync, "act": nc.scalar, "dve": nc.vector}[
                    cfg["out_q"]
                ]
                q.dma_start(out_d[:, cf0 : cf0 + sz], ot[:, 0:sz])

            for j in range(1, 1 + cfg["prefetch"]):
                fetch(j)
            grps = {}
            mms = {}
            for b in range(NB):
                fetch(b + 1 + cfg["prefetch"])
                if b not in slabs_r:
                    fetch(b)
                grps[b], mm = produce(b)
                if mm is not None:
                    mms[b] = mm
                if b - 1 in mms:
                    mms.pop(b - 1)()
                if b - 1 in grps and b - 1 not in mms:
                    evict_blk(b - 1, grps.pop(b - 1))
                if b - 2 in ots:
                    flush(b - 2)
            if NB - 1 in mms:
                mms.pop(NB - 1)()
            for b in (NB - 2, NB - 1):
                if b in grps:
                    evict_blk(b, grps.pop(b))
            for b in (NB - 3, NB - 2, NB - 1):
                if b in ots:
                    flush(b)

    _split_excess_waits(nc, 1)
    return nc


# --------------------------------------------------------------------------
# public entry point
# --------------------------------------------------------------------------
def kernel(position, coeffs):
    global LAST_RESULTS
    from concourse.bass_utils import run_bass_kernel_spmd

    position = np.asarray(position, dtype=np.float32)
    coeffs = np.asarray(coeffs, dtype=np.float32)
    OutN, n, _ = position.shape
    rows = OutN // N_CORES

    key = coeffs.tobytes()
    if key not in _BUILD_CACHE:
        _BUILD_CACHE[key] = _build_nc(_derive_terms(coeffs))
    nc = _BUILD_CACHE[key]

    wts = _build_wts_plane(_derive_terms(coeffs))
    blk_sizes = DEFAULT_CFG["blk_sizes"]
    b0 = blk_sizes[0]

    pos16 = position.astype(np.float16)
    in_maps = []
    for c in range(N_CORES):
        shard = pos16[c * rows : (c + 1) * rows]  # (rows, n, 3)
        planes = np.stack(
            [
                shard[..., 0].reshape(P, COLS),
                shard[..., 1].reshape(P, COLS),
                shard[..., 2].reshape(P, COLS),
            ],
            axis=1,
        )  # [P, 3, COLS]
        parts = []
        off = 0
        for sz in blk_sizes:
            parts.append(planes[:, :, off : off + sz].reshape(P, 3 * sz))
            off += sz
        in_maps.append(
            {
                "pos": np.ascontiguousarray(np.concatenate(parts[1:], axis=1)),
                "wts": np.ascontiguousarray(
                    np.concatenate([parts[0], wts], axis=1)
                ),
            }
        )

    res = None
    last_exc = None
    for attempt in range(4):
        try:
            res = run_bass_kernel_spmd(nc, in_maps, core_ids=list(range(N_CORES)))
        except Exception as exc:  # wedged-device resilience: retry fresh
            last_exc = exc
            import time as _time

            _time.sleep(10)
            continue
        # guard against transient first-run corruption (seen once as NaNs
        # at a block boundary on a cold device): rerun instead of returning
        finite = all(
            np.isfinite(res.results[c]["out"].astype(np.float32)).all()
            for c in range(N_CORES)
        )
        if finite:
            break
        last_exc = RuntimeError("non-finite kernel output")
        res = None
    if res is None:
        raise last_exc
    LAST_RESULTS = res
    out = np.empty((OutN, n), dtype=np.float32)
    for c in range(N_CORES):
        out[c * rows : (c + 1) * rows] = (
            res.results[c]["out"].astype(np.float32).reshape(rows, n)
        )
    return out
